# revision 22
# baseline (speedup 1.0000x reference)
"""Trainium2 Bass kernel for nn_BAESNN (STDP spiking net), 8-core data parallel.

Math (derived from the reference nn.Module):
  W0 = 6*I -> out_m = (6*x1 > 0.5)          elementwise spikes [B,40]
  W3 = 6*I -> out_p = (6*x2 > 0.5)          [B,40]
  W1 blocks of 2.0, W2 = 0:
      out_ifg[:, 0:20]  = a := any(out_m[:, 0:20])   (= 6*max(x1[:,0:20]) > 0.5)
      out_ifg[:, 20:30] = c := a | b
      out_ifg[:, 30:50] = b := any(out_m[:, 20:40])
  W4 blocks of 2.0, W5 = 0:
      out_sma = [a x5 | b x5] = out_ifg[:, [0:5, 30:35]];  W6=6*I -> out_m1 = out_sma
  dw_p_i = (out_m.T @ out_ifg, out_p.T @ out_ifg)  -> columns are copies of
      S_m[s, r] = sum_b spk_m[b, r] * stat_s[b], s in {a, b, c}   (same for p)
  The S stats are accumulated on the TensorEngine: per group of 8 "rows"
  (j-positions), stationary = [a|b|c] x 8 cols (bf16 0/1, exact), moving =
  spike block (bf16 0/1), PSUM f32. All sums are exact small integers.

The comparison (round_f32(6*x) > 0.5) is monotone in x, so it equals
(x >= X0) for the smallest f32 X0 crossing the boundary -> a single-op
is_ge tensor_scalar, bit-exact vs the reference matmul+compare.

Layout: each core gets R = B/8 rows. A chunk of 128*K rows is DMAed as a
contiguous [128, K*40] SBUF tile (partition p holds K consecutive rows), which
keeps every DMA fully contiguous and multiple MiB.
"""

import sys

sys.path.insert(0, "/opt/trn_rl_repo")

from contextlib import ExitStack

import numpy as np

import concourse.bacc as bacc
import concourse.bass as bass
import concourse.tile as tile
from concourse import mybir
from concourse.bass_utils import run_bass_kernel_spmd

F_IN = 40
F_IFG = 50
F_SMA = 10
N_CORES = 8
GJ = 8  # j-positions per matmul group
BF16_OUT = False  # store out_ifg as bf16 in DRAM (0/1 exact), cast to f32 on host

_nc_cache = {}


def _spike_cutoff():
    """Smallest f32 x with f32(6*x) > 0.5 (monotone boundary)."""
    x = np.float32(1.0 / 12.0)
    six = np.float32(6.0)
    half = np.float32(0.5)
    while six * x > half:
        x = np.nextafter(x, np.float32(0.0))
    while not (six * x > half):
        x = np.nextafter(x, np.float32(1.0))
    return float(x)


X0 = _spike_cutoff()


def build(R, K, nchunks, debug=False):
    """Build the single-core graph for a shard of R rows = 128 * K * nchunks."""
    assert R == 128 * K * nchunks
    assert K % GJ == 0
    G = K // GJ  # matmul groups per chunk
    f32 = mybir.dt.float32
    bf16 = mybir.dt.bfloat16

    nc = bacc.Bacc(
        "TRN2", target_bir_lowering=False, debug=debug, num_devices=N_CORES
    )

    x1 = nc.dram_tensor("x1", [R, F_IN], f32, kind="ExternalInput").ap()
    x2 = nc.dram_tensor("x2", [R, F_IN], f32, kind="ExternalInput").ap()
    ifg = nc.dram_tensor("ifg", [R, F_IFG], bf16 if BF16_OUT else f32, kind="ExternalOutput").ap()
    dw0 = nc.dram_tensor("dw0", [3 * GJ, 2 * GJ * F_IN], f32, kind="ExternalOutput").ap()
    dw1 = nc.dram_tensor("dw1", [3 * GJ, 2 * GJ * F_IN], f32, kind="ExternalOutput").ap()

    # rows r = c*(128K) + p*K + q  ->  [c, p, (q f)] contiguous per partition
    x1v = x1.rearrange("(c p q) f -> c p (q f)", c=nchunks, p=128)
    x2v = x2.rearrange("(c p q) f -> c p (q f)", c=nchunks, p=128)
    ifgv = ifg.rearrange("(c p q) f -> c p (q f)", c=nchunks, p=128)

    TS = mybir.AluOpType
    NF = GJ * F_IN  # moving columns per matmul

    with tile.TileContext(nc) as tc:
        with ExitStack() as ctx:
            pin = ctx.enter_context(tc.tile_pool(name="pin", bufs=5))
            pw = ctx.enter_context(tc.tile_pool(name="pw", bufs=2))
            pout = ctx.enter_context(tc.tile_pool(name="pout", bufs=3))
            pp = ctx.enter_context(tc.tile_pool(name="pp", bufs=1, space="PSUM"))
            pdw = ctx.enter_context(tc.tile_pool(name="pdw", bufs=1))

            # two psum generations: chunks [0, nsplit) drain early so only the
            # final chunk's tiny accumulator sits on the critical tail
            nsplit = max(nchunks - 1, 1)
            psum_m0 = pp.tile([3 * GJ, NF], f32)
            psum_p0 = pp.tile([3 * GJ, NF], f32)
            psum_m1 = pp.tile([3 * GJ, NF], f32)
            psum_p1 = pp.tile([3 * GJ, NF], f32)

            for c in range(nchunks):
                cx1 = pin.tile([128, K, F_IN], f32, tag="cx1")
                nc.sync.dma_start(
                    out=cx1[:].rearrange("p a b -> p (a b)"), in_=x1v[c]
                )
                cx2 = pin.tile([128, K, F_IN], f32, tag="cx2")
                nc.scalar.dma_start(
                    out=cx2[:].rearrange("p a b -> p (a b)"), in_=x2v[c]
                )

                # elementwise spikes (DVE, flat contiguous, single-op is_ge)
                spkm = pw.tile([128, K, F_IN], bf16, tag="spkm")
                nc.vector.tensor_scalar(
                    out=spkm[:].rearrange("p a b -> p (a b)"),
                    in0=cx1[:].rearrange("p a b -> p (a b)"),
                    scalar1=X0, scalar2=None, op0=TS.is_ge,
                )
                spkp = pw.tile([128, K, F_IN], bf16, tag="spkp")
                nc.vector.tensor_scalar(
                    out=spkp[:].rearrange("p a b -> p (a b)"),
                    in0=cx2[:].rearrange("p a b -> p (a b)"),
                    scalar1=X0, scalar2=None, op0=TS.is_ge,
                )

                # per-row half maxima of x1, stat-major layout [128, 2, K] (f32!)
                mx = pw.tile([128, 2, K], f32, tag="mx")
                nc.vector.tensor_reduce(
                    out=mx[:].rearrange("p h j -> p j h"),
                    in_=cx1[:].rearrange("p j (h e) -> p j h e", h=2),
                    axis=mybir.AxisListType.X,
                    op=TS.max,
                )
                # stats tile [128, 3, K]: rows a | b | c (contiguous per stat)
                st = pw.tile([128, 3, K], bf16, tag="st")
                nc.vector.tensor_scalar(
                    out=st[:, 0:2, :], in0=mx[:],
                    scalar1=X0, scalar2=None, op0=TS.is_ge,
                )
                nc.vector.tensor_max(st[:, 2, :], st[:, 0, :], st[:, 1, :])
                # group-major copy for matmul stationary operands:
                # st5[p, g, s, d] = st[p, s, g*GJ+d] -> 24 contiguous cols/group
                st5 = pw.tile([128, G, 3, GJ], bf16, tag="st5")
                nc.vector.tensor_copy(
                    out=st5[:],
                    in_=st[:].rearrange("p s (g d) -> p g s d", d=GJ),
                )

                # build out_ifg = [a x20 | c x10 | b x20]   (ScalarE copies)
                ifg_t = pout.tile([128, K, F_IFG], bf16 if BF16_OUT else f32, tag="ifg")
                a_b = st[:, 0, :].unsqueeze(-1)
                b_b = st[:, 1, :].unsqueeze(-1)
                c_b = st[:, 2, :].unsqueeze(-1)
                nc.scalar.copy(
                    out=ifg_t[:, :, 0:20], in_=a_b.broadcast_to((128, K, 20))
                )
                nc.scalar.copy(
                    out=ifg_t[:, :, 20:30], in_=c_b.broadcast_to((128, K, 10))
                )
                nc.scalar.copy(
                    out=ifg_t[:, :, 30:50], in_=b_b.broadcast_to((128, K, 20))
                )

                # dw stats: psum[GJ*s+d, 40d+f] += sum_p st[p,s,GJ*t+d] * spk[p,GJ*t+d,f]
                spkm_f = spkm[:].rearrange("p j f -> p (j f)")
                spkp_f = spkp[:].rearrange("p j f -> p (j f)")
                gen = 0 if c < nsplit else 1
                psum_m = psum_m0 if gen == 0 else psum_m1
                psum_p = psum_p0 if gen == 0 else psum_p1
                c0, cN = (0, nsplit) if gen == 0 else (nsplit, nchunks)
                for t in range(G):
                    first = c == c0 and t == 0
                    last = c == cN - 1 and t == G - 1
                    lhsT = st5[:, t, :, :].rearrange("p s d -> p (s d)")  # m = s*GJ+d
                    nc.tensor.matmul(
                        psum_m[:], lhsT, spkm_f[:, NF * t : NF * (t + 1)],
                        start=first, stop=last,
                    )
                    nc.tensor.matmul(
                        psum_p[:], lhsT, spkp_f[:, NF * t : NF * (t + 1)],
                        start=first, stop=last,
                    )
                if c == nsplit - 1 and nchunks > 1:
                    # drain generation 0 while the last chunk computes
                    dws0 = pdw.tile([3 * GJ, 2 * NF], f32, tag="dws0")
                    nc.vector.tensor_copy(out=dws0[:, 0:NF], in_=psum_m0[:])
                    nc.vector.tensor_copy(out=dws0[:, NF : 2 * NF], in_=psum_p0[:])
                    nc.gpsimd.dma_start(out=dw0, in_=dws0[:])

                # output DMA alternating between the two HWDGE rings
                out_eng = nc.sync if c % 2 else nc.scalar
                out_eng.dma_start(
                    out=ifgv[c], in_=ifg_t[:].rearrange("p a b -> p (a b)")
                )

            dws1 = pdw.tile([3 * GJ, 2 * NF], f32, tag="dws1")
            if nchunks > 1:
                nc.vector.tensor_copy(out=dws1[:, 0:NF], in_=psum_m1[:])
                nc.vector.tensor_copy(out=dws1[:, NF : 2 * NF], in_=psum_p1[:])
            else:
                nc.vector.memset(dws1[:], 0.0)
            nc.gpsimd.dma_start(out=dw1, in_=dws1[:])

    nc.compile()
    return nc


def _get_nc(R, K, nchunks):
    key = (R, K, nchunks)
    if key not in _nc_cache:
        _nc_cache[key] = build(R, K, nchunks)
    return _nc_cache[key]


def _assemble(results, R_total):
    """Host-side gather/unshard: concat batch shards, reduce dw partials."""
    ifg = np.concatenate([r["ifg"] for r in results], axis=0)
    if ifg.dtype != np.float32:
        ifg = ifg.astype(np.float32)
    # out_sma = [a x5 | b x5]; a is ifg cols 0:20, b is cols 30:50 (device-computed)
    sma = np.concatenate([ifg[:, 0:5], ifg[:, 30:35]], axis=1)
    m1 = sma.copy()

    NF = GJ * F_IN
    P = np.zeros((3 * GJ, 2 * NF), dtype=np.float64)
    for r in results:
        P += r["dw0"].astype(np.float64)
        P += r["dw1"].astype(np.float64)
    Pm = P[:, :NF].reshape(3, GJ, GJ, F_IN)  # [s, d, d', f]
    Pp = P[:, NF:].reshape(3, GJ, GJ, F_IN)
    S_m = np.einsum("sddf->sf", Pm)  # [3, 40] stats {a,b,c} x feature
    S_p = np.einsum("sddf->sf", Pp)

    def expand_ifg(S):
        return np.concatenate(
            [
                np.repeat(S[0][:, None], 20, axis=1),
                np.repeat(S[2][:, None], 10, axis=1),
                np.repeat(S[1][:, None], 20, axis=1),
            ],
            axis=1,
        ).astype(np.float32)

    def expand_sma(S):
        return np.concatenate(
            [
                np.repeat(S[0][:, None], 5, axis=1),
                np.repeat(S[1][:, None], 5, axis=1),
            ],
            axis=1,
        ).astype(np.float32)

    dw_p_i0 = expand_ifg(S_m)
    dw_p_i1 = expand_ifg(S_p)
    dw_p_s0 = expand_sma(S_m)
    dw_p_s1 = expand_sma(S_p)
    return (dw_p_i0, dw_p_i1, dw_p_s0, dw_p_s1, ifg, sma, m1)


def kernel(**inputs):
    x1 = np.ascontiguousarray(np.asarray(inputs["x1"], dtype=np.float32))
    x2 = np.ascontiguousarray(np.asarray(inputs["x2"], dtype=np.float32))
    B = x1.shape[0]
    assert B % N_CORES == 0
    R = B // N_CORES
    K = 64
    while K > GJ and R % (128 * K) != 0:
        K //= 2
    assert R % (128 * K) == 0, f"unsupported shard size {R}"
    nchunks = R // (128 * K)

    nc = _get_nc(R, K, nchunks)

    in_maps = [
        {
            "x1": x1[i * R : (i + 1) * R],
            "x2": x2[i * R : (i + 1) * R],
        }
        for i in range(N_CORES)
    ]
    res = run_bass_kernel_spmd(nc, in_maps, list(range(N_CORES)))
    return _assemble(res.results, B)


if __name__ == "__main__":
    rng = np.random.default_rng(0)
    B = 128 * 8 * 2 * N_CORES
    x1 = rng.random((B, F_IN), dtype=np.float32)
    x2 = rng.random((B, F_IN), dtype=np.float32)
    out = kernel(x1=x1, x2=x2)
    print([o.shape for o in out])


# revision 24
# speedup vs baseline: 1.4093x; 1.4093x over previous
"""Trainium2 Bass kernel for nn_BAESNN (STDP spiking net), 8-core data parallel.

Math (derived from the reference nn.Module):
  W0 = 6*I -> out_m = (6*x1 > 0.5)          elementwise spikes [B,40]
  W3 = 6*I -> out_p = (6*x2 > 0.5)          [B,40]
  W1 blocks of 2.0, W2 = 0:
      out_ifg[:, 0:20]  = a := any(out_m[:, 0:20])   (= 6*max(x1[:,0:20]) > 0.5)
      out_ifg[:, 20:30] = c := a | b
      out_ifg[:, 30:50] = b := any(out_m[:, 20:40])
  W4 blocks of 2.0, W5 = 0:
      out_sma = [a x5 | b x5] = out_ifg[:, [0:5, 30:35]];  W6=6*I -> out_m1 = out_sma
  dw_p_i = (out_m.T @ out_ifg, out_p.T @ out_ifg)  -> columns are copies of
      S_m[s, r] = sum_b spk_m[b, r] * stat_s[b], s in {a, b, c}   (same for p)
  The S stats are accumulated on the TensorEngine: per group of 8 "rows"
  (j-positions), stationary = [a|b|c] x 8 cols (bf16 0/1, exact), moving =
  spike block (bf16 0/1), PSUM f32. All sums are exact small integers.

The comparison (round_f32(6*x) > 0.5) is monotone in x, so it equals
(x >= X0) for the smallest f32 X0 crossing the boundary -> a single-op
is_ge tensor_scalar, bit-exact vs the reference matmul+compare.

Layout: each core gets R = B/8 rows. A chunk of 128*K rows is DMAed as a
contiguous [128, K*40] SBUF tile (partition p holds K consecutive rows), which
keeps every DMA fully contiguous and multiple MiB.
"""

import sys

sys.path.insert(0, "/opt/trn_rl_repo")

from contextlib import ExitStack

import numpy as np

import concourse.bacc as bacc
import concourse.bass as bass
import concourse.tile as tile
from concourse import mybir
from concourse.bass_utils import run_bass_kernel_spmd

F_IN = 40
F_IFG = 50
F_SMA = 10
N_CORES = 8
GJ = 8  # j-positions per matmul group
BF16_OUT = True  # store out_ifg as bf16 in DRAM (values are exactly 0/1 so
# bf16 is lossless); host casts back to f32. Set False for pure-f32 DRAM IO
# (~175us vs ~150us).

_nc_cache = {}


def _spike_cutoff():
    """Smallest f32 x with f32(6*x) > 0.5 (monotone boundary)."""
    x = np.float32(1.0 / 12.0)
    six = np.float32(6.0)
    half = np.float32(0.5)
    while six * x > half:
        x = np.nextafter(x, np.float32(0.0))
    while not (six * x > half):
        x = np.nextafter(x, np.float32(1.0))
    return float(x)


X0 = _spike_cutoff()


def build(R, K, nchunks, debug=False):
    """Build the single-core graph for a shard of R rows = 128 * K * nchunks."""
    assert R == 128 * K * nchunks
    assert K % GJ == 0
    G = K // GJ  # matmul groups per chunk
    f32 = mybir.dt.float32
    bf16 = mybir.dt.bfloat16

    nc = bacc.Bacc(
        "TRN2", target_bir_lowering=False, debug=debug, num_devices=N_CORES
    )

    x1 = nc.dram_tensor("x1", [R, F_IN], f32, kind="ExternalInput").ap()
    x2 = nc.dram_tensor("x2", [R, F_IN], f32, kind="ExternalInput").ap()
    ifg = nc.dram_tensor("ifg", [R, F_IFG], bf16 if BF16_OUT else f32, kind="ExternalOutput").ap()
    dw0 = nc.dram_tensor("dw0", [3 * GJ, 2 * GJ * F_IN], f32, kind="ExternalOutput").ap()
    dw1 = nc.dram_tensor("dw1", [3 * GJ, 2 * GJ * F_IN], f32, kind="ExternalOutput").ap()

    # rows r = c*(128K) + p*K + q  ->  [c, p, (q f)] contiguous per partition
    x1v = x1.rearrange("(c p q) f -> c p (q f)", c=nchunks, p=128)
    x2v = x2.rearrange("(c p q) f -> c p (q f)", c=nchunks, p=128)
    ifgv = ifg.rearrange("(c p q) f -> c p (q f)", c=nchunks, p=128)

    TS = mybir.AluOpType
    NF = GJ * F_IN  # moving columns per matmul

    with tile.TileContext(nc) as tc:
        with ExitStack() as ctx:
            pin = ctx.enter_context(tc.tile_pool(name="pin", bufs=4))
            pw = ctx.enter_context(tc.tile_pool(name="pw", bufs=2))
            pout = ctx.enter_context(tc.tile_pool(name="pout", bufs=3))
            pp = ctx.enter_context(tc.tile_pool(name="pp", bufs=1, space="PSUM"))
            pdw = ctx.enter_context(tc.tile_pool(name="pdw", bufs=1))

            # two psum generations: chunks [0, nsplit) drain early so only the
            # final chunk's tiny accumulator sits on the critical tail
            nsplit = max(nchunks - 1, 1)
            psum_m0 = pp.tile([3 * GJ, NF], f32)
            psum_p0 = pp.tile([3 * GJ, NF], f32)
            psum_m1 = pp.tile([3 * GJ, NF], f32)
            psum_p1 = pp.tile([3 * GJ, NF], f32)

            for c in range(nchunks):
                cx1 = pin.tile([128, K, F_IN], f32, tag="cx1")
                nc.sync.dma_start(
                    out=cx1[:].rearrange("p a b -> p (a b)"), in_=x1v[c]
                )
                cx2 = pin.tile([128, K, F_IN], f32, tag="cx2")
                nc.scalar.dma_start(
                    out=cx2[:].rearrange("p a b -> p (a b)"), in_=x2v[c]
                )

                # elementwise spikes (DVE, flat contiguous, single-op is_ge)
                spkm = pw.tile([128, K, F_IN], bf16, tag="spkm")
                nc.vector.tensor_scalar(
                    out=spkm[:].rearrange("p a b -> p (a b)"),
                    in0=cx1[:].rearrange("p a b -> p (a b)"),
                    scalar1=X0, scalar2=None, op0=TS.is_ge,
                )
                spkp = pw.tile([128, K, F_IN], bf16, tag="spkp")
                nc.vector.tensor_scalar(
                    out=spkp[:].rearrange("p a b -> p (a b)"),
                    in0=cx2[:].rearrange("p a b -> p (a b)"),
                    scalar1=X0, scalar2=None, op0=TS.is_ge,
                )

                # per-row half maxima of x1, stat-major layout [128, 2, K] (f32!)
                mx = pw.tile([128, 2, K], f32, tag="mx")
                nc.vector.tensor_reduce(
                    out=mx[:].rearrange("p h j -> p j h"),
                    in_=cx1[:].rearrange("p j (h e) -> p j h e", h=2),
                    axis=mybir.AxisListType.X,
                    op=TS.max,
                )
                # stats tile [128, 3, K]: rows a | b | c (contiguous per stat)
                st = pw.tile([128, 3, K], bf16, tag="st")
                nc.vector.tensor_scalar(
                    out=st[:, 0:2, :], in0=mx[:],
                    scalar1=X0, scalar2=None, op0=TS.is_ge,
                )
                nc.vector.tensor_max(st[:, 2, :], st[:, 0, :], st[:, 1, :])
                # group-major copy for matmul stationary operands:
                # st5[p, g, s, d] = st[p, s, g*GJ+d] -> 24 contiguous cols/group
                st5 = pw.tile([128, G, 3, GJ], bf16, tag="st5")
                nc.vector.tensor_copy(
                    out=st5[:],
                    in_=st[:].rearrange("p s (g d) -> p g s d", d=GJ),
                )

                # build out_ifg = [a x20 | c x10 | b x20]   (ScalarE copies)
                ifg_t = pout.tile([128, K, F_IFG], bf16 if BF16_OUT else f32, tag="ifg")
                a_b = st[:, 0, :].unsqueeze(-1)
                b_b = st[:, 1, :].unsqueeze(-1)
                c_b = st[:, 2, :].unsqueeze(-1)
                nc.scalar.copy(
                    out=ifg_t[:, :, 0:20], in_=a_b.broadcast_to((128, K, 20))
                )
                nc.scalar.copy(
                    out=ifg_t[:, :, 20:30], in_=c_b.broadcast_to((128, K, 10))
                )
                nc.scalar.copy(
                    out=ifg_t[:, :, 30:50], in_=b_b.broadcast_to((128, K, 20))
                )

                # dw stats: psum[GJ*s+d, 40d+f] += sum_p st[p,s,GJ*t+d] * spk[p,GJ*t+d,f]
                spkm_f = spkm[:].rearrange("p j f -> p (j f)")
                spkp_f = spkp[:].rearrange("p j f -> p (j f)")
                gen = 0 if c < nsplit else 1
                psum_m = psum_m0 if gen == 0 else psum_m1
                psum_p = psum_p0 if gen == 0 else psum_p1
                c0, cN = (0, nsplit) if gen == 0 else (nsplit, nchunks)
                for t in range(G):
                    first = c == c0 and t == 0
                    last = c == cN - 1 and t == G - 1
                    lhsT = st5[:, t, :, :].rearrange("p s d -> p (s d)")  # m = s*GJ+d
                    nc.tensor.matmul(
                        psum_m[:], lhsT, spkm_f[:, NF * t : NF * (t + 1)],
                        start=first, stop=last,
                    )
                    nc.tensor.matmul(
                        psum_p[:], lhsT, spkp_f[:, NF * t : NF * (t + 1)],
                        start=first, stop=last,
                    )
                if c == nsplit - 1 and nchunks > 1:
                    # drain generation 0 while the last chunk computes
                    dws0 = pdw.tile([3 * GJ, 2 * NF], f32, tag="dws0")
                    nc.vector.tensor_copy(out=dws0[:, 0:NF], in_=psum_m0[:])
                    nc.vector.tensor_copy(out=dws0[:, NF : 2 * NF], in_=psum_p0[:])
                    nc.gpsimd.dma_start(out=dw0, in_=dws0[:])

                # output DMA alternating between the two HWDGE rings
                out_eng = nc.sync if c % 2 else nc.scalar
                out_eng.dma_start(
                    out=ifgv[c], in_=ifg_t[:].rearrange("p a b -> p (a b)")
                )

            dws1 = pdw.tile([3 * GJ, 2 * NF], f32, tag="dws1")
            if nchunks > 1:
                nc.vector.tensor_copy(out=dws1[:, 0:NF], in_=psum_m1[:])
                nc.vector.tensor_copy(out=dws1[:, NF : 2 * NF], in_=psum_p1[:])
            else:
                nc.vector.memset(dws1[:], 0.0)
            nc.gpsimd.dma_start(out=dw1, in_=dws1[:])

    nc.compile()
    return nc


def _get_nc(R, K, nchunks):
    key = (R, K, nchunks)
    if key not in _nc_cache:
        _nc_cache[key] = build(R, K, nchunks)
    return _nc_cache[key]


def _assemble(results, R_total):
    """Host-side gather/unshard: concat batch shards, reduce dw partials."""
    ifg = np.concatenate([r["ifg"] for r in results], axis=0)
    if ifg.dtype != np.float32:
        ifg = ifg.astype(np.float32)
    # out_sma = [a x5 | b x5]; a is ifg cols 0:20, b is cols 30:50 (device-computed)
    sma = np.concatenate([ifg[:, 0:5], ifg[:, 30:35]], axis=1)
    m1 = sma.copy()

    NF = GJ * F_IN
    P = np.zeros((3 * GJ, 2 * NF), dtype=np.float64)
    for r in results:
        P += r["dw0"].astype(np.float64)
        P += r["dw1"].astype(np.float64)
    Pm = P[:, :NF].reshape(3, GJ, GJ, F_IN)  # [s, d, d', f]
    Pp = P[:, NF:].reshape(3, GJ, GJ, F_IN)
    S_m = np.einsum("sddf->sf", Pm)  # [3, 40] stats {a,b,c} x feature
    S_p = np.einsum("sddf->sf", Pp)

    def expand_ifg(S):
        return np.concatenate(
            [
                np.repeat(S[0][:, None], 20, axis=1),
                np.repeat(S[2][:, None], 10, axis=1),
                np.repeat(S[1][:, None], 20, axis=1),
            ],
            axis=1,
        ).astype(np.float32)

    def expand_sma(S):
        return np.concatenate(
            [
                np.repeat(S[0][:, None], 5, axis=1),
                np.repeat(S[1][:, None], 5, axis=1),
            ],
            axis=1,
        ).astype(np.float32)

    dw_p_i0 = expand_ifg(S_m)
    dw_p_i1 = expand_ifg(S_p)
    dw_p_s0 = expand_sma(S_m)
    dw_p_s1 = expand_sma(S_p)
    return (dw_p_i0, dw_p_i1, dw_p_s0, dw_p_s1, ifg, sma, m1)


def kernel(**inputs):
    x1 = np.ascontiguousarray(np.asarray(inputs["x1"], dtype=np.float32))
    x2 = np.ascontiguousarray(np.asarray(inputs["x2"], dtype=np.float32))
    B = x1.shape[0]
    assert B % N_CORES == 0
    R = B // N_CORES
    K = 64
    while K > GJ and R % (128 * K) != 0:
        K //= 2
    assert R % (128 * K) == 0, f"unsupported shard size {R}"
    nchunks = R // (128 * K)

    nc = _get_nc(R, K, nchunks)

    in_maps = [
        {
            "x1": x1[i * R : (i + 1) * R],
            "x2": x2[i * R : (i + 1) * R],
        }
        for i in range(N_CORES)
    ]
    res = run_bass_kernel_spmd(nc, in_maps, list(range(N_CORES)))
    return _assemble(res.results, B)


if __name__ == "__main__":
    rng = np.random.default_rng(0)
    B = 128 * 8 * 2 * N_CORES
    x1 = rng.random((B, F_IN), dtype=np.float32)
    x2 = rng.random((B, F_IN), dtype=np.float32)
    out = kernel(x1=x1, x2=x2)
    print([o.shape for o in out])
